# revision 35
# baseline (speedup 1.0000x reference)
"""Trainium2 Bass kernel for nn_AttentiveSSM (sparse chunked attention + SSM).

Sharding (8 cores, tensor-parallel over heads):
  core c owns q-heads {2c, 2c+1} and kv-head c//2. Each core computes its
  Q/K/V projections from the full (transposed) x in bf16, runs the chunked
  SSM + RoPE, sparse attention against the compressed key set (chunk
  boundaries + first-4 + t-1 diagonal), and a partial output projection
  through its wo column slice. Host sums the 8 partial yT outputs.

All PE operands are bf16 (1 cycle/row, fast weight load); accumulation is
fp32 in PSUM. Scores are grouped (G0=[first4|chunk0], C1=chunk1,
G1=[chunk1|chunk2], G2=chunk3) so each score/PV matmul streams a full
512-row block. K/V histories carry a leading zero column so the t-1
diagonal has no block-edge special case.

Self-contained: hardcodes all shapes; no sibling imports.
"""
import sys
import numpy as np

sys.path.insert(0, '/opt/trn_rl_repo')

import concourse.bacc as bacc               # noqa: E402
import concourse.mybir as mybir             # noqa: E402
from concourse.tile import TileContext      # noqa: E402
from concourse import bass_utils            # noqa: E402
from concourse.alu_op_type import AluOpType # noqa: E402

# silence cloud artifact upload in traced runs
bass_utils.upload_artifacts = lambda tmpdir: tmpdir

S = 2048          # sequence
D = 2048          # model dim
HD = 128          # head dim
QB = 512          # query block
NKT = D // 128    # 16 contraction tiles
KC = 8            # token chunk
NEG = -1.0e9
SCALE = float(1.0 / np.sqrt(HD))

F32 = mybir.dt.float32
F32R = mybir.dt.float32r
BF16 = mybir.dt.bfloat16
MUL = AluOpType.mult
ADD = AluOpType.add
EXP = mybir.ActivationFunctionType.Exp

# f32c column layout
MG0B0, MG1B2, MG2B3, AKP, AVP = 0, 512, 1024, 1536, 2048
CBK, CBV, ONESF = 2560, 2561, 2562
F32C_W = 2690
# bf16c column layout (mdiag lives in row 0 as four 512-col blocks)
MDIAG, IDENT, ONES = 0, 2048, 2176
BF16C_W = 2304

# key-group layout inside KCt/VG (free dim):
#   [0:4] first4, [4:68] chunk0 -> G0 [0:68]
#   [68:132] chunk1 (C1), [132:196] chunk2 -> G1 [68:196]
#   [196:260] chunk3 -> G2
G0_LO, G0_N = 0, 68
C1_LO, C1_N = 68, 64
G1_LO, G1_N = 68, 128
G2_LO, G2_N = 196, 64

_CACHE = {}


def _build_module():
    nc = bacc.Bacc("TRN2", num_devices=8)

    def din(name, shape, dt):
        return nc.dram_tensor(name, list(shape), dt, kind="ExternalInput")

    xT = din("xT", (D, S), BF16)
    wq = din("wq", (128, NKT * 256), BF16)
    wk = din("wk", (128, NKT * 128), BF16)
    wv = din("wv", (128, NKT * 128), BF16)
    wo = din("wo", (128, 2 * D), BF16)
    cosk = din("cosk", (128, S), F32)
    sink = din("sink", (128, S), F32)
    f32c = din("f32c", (128, F32C_W), F32)
    bf16c = din("bf16c", (128, BF16C_W), BF16)
    # yT in blocked layout: [p, (sb*16 + dtile)*512 + c] = y[128*dtile+p,
    # 512*sb+c]; each wo pair writes one contiguous [128,1024] block
    yT = nc.dram_tensor("yT", [128, 4 * NKT * QB], BF16,
                        kind="ExternalOutput")

    with TileContext(nc) as tc:
        with (
            tc.tile_pool(name="const", bufs=1) as cp,
            tc.tile_pool(name="xs", bufs=16) as xs,
            tc.tile_pool(name="big", bufs=1) as bp,
            tc.tile_pool(name="tp", bufs=2) as tp,
            tc.tile_pool(name="pj", bufs=1, space="PSUM") as pj,
            tc.tile_pool(name="at", bufs=1, space="PSUM") as at,
        ):
            # ---- x tiles: [128,1024] spanning an sblock PAIR per ktile
            # (2KB per-partition lines -> full DMA bandwidth), issues split
            # across the sync and gpsimd rings. The second pair is emitted
            # after proj 0/1 so slot-reuse waits cannot stall anything that
            # matters. ----
            xt = {}

            def load_x(pair):
                for k in range(NKT):
                    t = xs.tile([128, 2 * QB], BF16, tag="x",
                                name=f"x{pair}_{k}")
                    eng = nc.sync if k % 2 == 0 else nc.gpsimd
                    eng.dma_start(t[:], xT[128 * k:128 * (k + 1),
                                           2 * QB * pair:2 * QB * (pair + 1)])
                    xt[(pair, k)] = t

            def xtile(sb, k):
                return xt[(sb // 2, k)][:, QB * (sb % 2):QB * (sb % 2 + 1)]

            # issue order front-loads proj0's data (wq, x sb0); the bulky
            # late-use constants (cos/sin/fc/bc/wo) go behind x sb0/sb1 so
            # the first matmuls aren't stuck behind ~8MB of DMA.
            wq_sb = cp.tile([128, NKT * 256], BF16, tag="wq")
            # first wq half rides the otherwise-idle scalar ring so it lands
            # in parallel with the first x tiles on sync/gpsimd
            nc.scalar.dma_start(wq_sb[:, 0:NKT * 128], wq[:, 0:NKT * 128])
            load_x(0)
            nc.scalar.dma_start(wq_sb[:, NKT * 128:NKT * 256],
                                wq[:, NKT * 128:NKT * 256])
            wk_sb = cp.tile([128, NKT * 128], BF16, tag="wk")
            nc.sync.dma_start(wk_sb[:], wk[:])
            wv_sb = cp.tile([128, NKT * 128], BF16, tag="wv")
            nc.sync.dma_start(wv_sb[:], wv[:])
            cos_sb = cp.tile([128, S], F32, tag="cos")
            nc.sync.dma_start(cos_sb[:], cosk[:])
            sin_sb = cp.tile([128, S], F32, tag="sin")
            nc.sync.dma_start(sin_sb[:], sink[:])
            fc = cp.tile([128, F32C_W], F32, tag="f32c")
            nc.sync.dma_start(fc[:], f32c[:])
            bc = cp.tile([128, BF16C_W], BF16, tag="bf16c")
            nc.sync.dma_start(bc[:], bf16c[:])
            wo_sb = cp.tile([128, 2 * D], BF16, tag="wo")
            nc.sync.dma_start(wo_sb[:], wo[:])

            # ---- big state ----
            QT0 = bp.tile([128, S], BF16, tag="QT0")
            QT1 = bp.tile([128, S], BF16, tag="QT1")
            KTx = bp.tile([128, S + 1], BF16, tag="KTx")   # col0 = 0 pad
            VTx = bp.tile([128, S + 1], BF16, tag="VTx")
            OT0 = bp.tile([128, S], BF16, tag="OT0")
            OT1 = bp.tile([128, S], BF16, tag="OT1")
            KCt = bp.tile([128, 260], BF16, tag="KCt")
            VG = bp.tile([128, 260], BF16, tag="VG")
            vcG0 = bp.tile([G0_N, 128], BF16, tag="vcG0")
            vcC1 = bp.tile([C1_N, 128], BF16, tag="vcC1")
            vcG1 = bp.tile([G1_N, 128], BF16, tag="vcG1")
            vcG2 = bp.tile([G2_N, 128], BF16, tag="vcG2")
            VCS = {'G0': vcG0, 'C1': vcC1, 'G1': vcG1, 'G2': vcG2}

            nc.vector.memset(KTx[:, 0:1], 0)
            nc.vector.memset(VTx[:, 0:1], 0)

            def rope(label, src, s0, ueng=None):
                # t + u = src*cos2 + swap(src)*sin2 ; sin2 = [sin; -sin]
                ueng = ueng or nc.vector
                t = tp.tile([128, QB], F32, tag="ropet", name=f"rt{label}")
                u = tp.tile([128, QB], F32, tag="ropeu", name=f"ru{label}")
                nc.vector.tensor_tensor(t[:], src[:], cos_sb[:, s0:s0 + QB], MUL)
                ueng.tensor_tensor(u[0:64, :], src[64:128, :],
                                   sin_sb[64:128, s0:s0 + QB], MUL)
                ueng.tensor_tensor(u[64:128, :], src[0:64, :],
                                   sin_sb[0:64, s0:s0 + QB], MUL)
                return t, u

            def proj_sblock(sb):
                s0 = QB * sb
                qq = pj.tile([128, 2 * QB], F32, tag="qq", name=f"qq_{sb}")
                kv = pj.tile([128, 2 * QB], F32, tag="kv", name=f"kv_{sb}")
                psq0 = qq[:, 0:QB]
                psq1 = qq[:, QB:2 * QB]
                psk = kv[:, 0:QB]
                psv = kv[:, QB:2 * QB]
                for half in range(2):
                    ks = [half * 8 + kk for kk in range(8)]
                    for k in ks:
                        nc.tensor.matmul(psq0, wq_sb[:, k * 256:k * 256 + 128],
                                         xtile(sb, k), start=(k == 0),
                                         stop=(k == NKT - 1))
                    for k in ks:
                        nc.tensor.matmul(psq1,
                                         wq_sb[:, k * 256 + 128:k * 256 + 256],
                                         xtile(sb, k), start=(k == 0),
                                         stop=(k == NKT - 1))
                    for k in ks:
                        nc.tensor.matmul(psk, wk_sb[:, k * 128:(k + 1) * 128],
                                         xtile(sb, k), start=(k == 0),
                                         stop=(k == NKT - 1))
                    for k in ks:
                        nc.tensor.matmul(psv, wv_sb[:, k * 128:(k + 1) * 128],
                                         xtile(sb, k), start=(k == 0),
                                         stop=(k == NKT - 1))
                # Q0 rope first; the scalar-engine staging copy frees the
                # PSUM slice immediately (direct-PSUM rope would hold the
                # bank hostage to the backlogged DVE queue)
                q0s = tp.tile([128, QB], F32, tag="q0s", name=f"q0s{sb}")
                nc.scalar.copy(q0s[:], psq0)
                t, u = rope(f"q0_{sb}", q0s, s0)
                nc.vector.tensor_tensor(QT0[:, s0:s0 + QB], t[:], u[:], ADD)
                # K chain: SSM -> rope -> boundary gather
                hk = tp.tile([128, QB], F32, tag="hk", name=f"hk{sb}")
                nc.vector.tensor_tensor_scan(hk[:], fc[:, AKP:AKP + QB], psk,
                                             0.0, MUL, ADD)
                kp = tp.tile([128, QB], F32, tag="kp", name=f"kp{sb}")
                nc.vector.scalar_tensor_tensor(kp[:], hk[:], fc[:, CBK:CBK + 1],
                                               psk, MUL, ADD)
                t, u = rope(f"k_{sb}", kp, s0)
                nc.vector.tensor_tensor(KTx[:, 1 + s0:1 + s0 + QB], t[:], u[:],
                                        ADD)
                nc.gpsimd.tensor_copy(KCt[:, 4 + 64 * sb:4 + 64 * (sb + 1)],
                                      KTx[:, s0 + 8:s0 + 513:8])
                if sb == 0:
                    nc.gpsimd.tensor_copy(KCt[:, 0:4], KTx[:, 1:5])
                # Q1 rope
                q1s = tp.tile([128, QB], F32, tag="q1s", name=f"q1s{sb}")
                nc.scalar.copy(q1s[:], psq1)
                t, u = rope(f"q1_{sb}", q1s, s0)
                nc.vector.tensor_tensor(QT1[:, s0:s0 + QB], t[:], u[:], ADD)
                # V chain on scalar+gpsimd (keeps DVE free for K/Q):
                # copy psv out of PSUM, then SSM + gather off-DVE
                vsb = tp.tile([128, QB], F32, tag="vsb", name=f"vsb{sb}")
                nc.scalar.copy(vsb[:], psv)
                hv = tp.tile([128, QB], F32, tag="hv", name=f"hv{sb}")
                nc.vector.tensor_tensor_scan(hv[:], fc[:, AVP:AVP + QB],
                                             vsb[:], 0.0, MUL, ADD)
                nc.vector.scalar_tensor_tensor(VTx[:, 1 + s0:1 + s0 + QB],
                                               hv[:], fc[:, CBV:CBV + 1],
                                               vsb[:], MUL, ADD)
                nc.gpsimd.tensor_copy(VG[:, 4 + 64 * sb:4 + 64 * (sb + 1)],
                                      VTx[:, s0 + 8:s0 + 513:8])
                if sb == 0:
                    nc.gpsimd.tensor_copy(VG[:, 0:4], VTx[:, 1:5])

            def vtrans(sb):
                dst, lo, n = [(vcG0, G0_LO, G0_N), (vcC1, C1_LO, C1_N),
                              (vcG1, G1_LO, G1_N), (vcG2, G2_LO, G2_N)][sb]
                pst = at.tile([n, 128], BF16, tag="st", bufs=2,
                              name=f"tr{sb}")
                nc.tensor.transpose(pst[:], VG[:, lo:lo + n],
                                    bc[:, IDENT:IDENT + 128])
                nc.scalar.copy(dst[:], pst[:])

            # per q-block key groups: (name, lo, n, mask (rows, col) or None)
            GROUPS = [
                [('G0', G0_LO, G0_N, (0, MG0B0))],
                [('G0', G0_LO, G0_N, None), ('C1', C1_LO, C1_N, (0, MG2B3))],
                [('G0', G0_LO, G0_N, None), ('G1', G1_LO, G1_N, (0, MG1B2))],
                [('G0', G0_LO, G0_N, None), ('G1', G1_LO, G1_N, None),
                 ('G2', G2_LO, G2_N, (0, MG2B3))],
            ]

            def attn_scores(b, h):
                q0 = QB * b
                QTh = QT0 if h == 0 else QT1
                plist = []
                for (gn, lo, n, mk) in GROUPS[b]:
                    st = at.tile([n, QB], F32, tag="st", bufs=2,
                                 name=f"st{b}{h}{gn}")
                    nc.tensor.matmul(st[:], KCt[:, lo:lo + n],
                                     QTh[:, q0:q0 + QB], start=True, stop=True)
                    if mk is not None:
                        r0, mc = mk
                        nc.vector.tensor_tensor(st[:], st[:],
                                                fc[r0:r0 + n, mc:mc + QB], ADD)
                    P = tp.tile([n, QB], BF16, tag="P", bufs=6,
                                name=f"P{b}{h}{gn}")
                    nc.scalar.activation(P[:], st[:], EXP, scale=SCALE)
                    plist.append((P, n, VCS[gn]))
                # t-1 diagonal: z = q . k_shifted, col-summed on PE
                z = tp.tile([128, QB], BF16, tag="z", name=f"z{b}{h}")
                nc.vector.tensor_tensor(z[:], QTh[:, q0:q0 + QB],
                                        KTx[:, q0:q0 + QB], MUL)
                sd = at.tile([1, QB], F32, tag="sm", name=f"sd{b}{h}")
                nc.tensor.matmul(sd[:], bc[:, ONES:ONES + 1],
                                 z[:], start=True, stop=False)
                nc.tensor.matmul(sd[:], bc[0:1, ONES:ONES + 1],
                                 bc[0:1, MDIAG + QB * b:MDIAG + QB * (b + 1)],
                                 start=False, stop=True)
                pd = tp.tile([1, QB], BF16, tag="pd", bufs=4, name=f"pd{b}{h}")
                nc.scalar.activation(pd[:], sd[:], EXP, scale=SCALE)
                return plist, pd

            def attn_tail(b, h, plist, pd):
                q0 = QB * b
                OTh = OT0 if h == 0 else OT1
                dent = at.tile([1, QB], F32, tag="sm", name=f"den{b}{h}")
                den = dent[:]
                oun = at.tile([128, QB], F32, tag="oun", name=f"oun{b}{h}")
                npl = len(plist)
                for i, (P, n, _) in enumerate(plist):
                    nc.tensor.matmul(den, bc[0:n, ONES:ONES + 1], P[:],
                                     start=(i == 0), stop=False)
                # diagonal V term: vsh runs on DVE while oun matmuls stream
                psb = at.tile([128, QB], F32, tag="st", bufs=2,
                              name=f"psb{b}{h}")
                nc.tensor.matmul(psb[:], bc[0:1, ONES:ONES + 128], pd[:],
                                 start=True, stop=True)
                vsh = tp.tile([128, QB], F32, tag="vsh", name=f"vsh{b}{h}")
                nc.vector.tensor_tensor(vsh[:], psb[:], VTx[:, q0:q0 + QB], MUL)
                nc.tensor.matmul(den, bc[0:1, ONES:ONES + 1], pd[:],
                                 start=False, stop=True)
                for i, (P, n, vt) in enumerate(plist):
                    nc.tensor.matmul(oun[:], vt[:], P[:],
                                     start=(i == 0), stop=(i == npl - 1))
                nc.vector.tensor_tensor(oun[:], oun[:], vsh[:], ADD)
                # normalize
                rec = tp.tile([1, QB], F32, tag="rec", bufs=4,
                              name=f"rec{b}{h}")
                nc.vector.reciprocal_approx_fast(rec[:], den)
                recb = tp.tile([1, QB], BF16, tag="recb", bufs=4,
                               name=f"recb{b}{h}")
                nc.scalar.copy(recb[:], rec[:])
                rb = at.tile([128, QB], F32, tag="st", bufs=2, name=f"rb{b}{h}")
                nc.tensor.matmul(rb[:], bc[0:1, ONES:ONES + 128], recb[:],
                                 start=True, stop=True)
                rbs = tp.tile([128, QB], F32, tag="rbs", name=f"rbs{b}{h}")
                nc.scalar.copy(rbs[:], rb[:])
                nc.vector.tensor_tensor(OTh[:, q0:q0 + QB], oun[:], rbs[:], MUL)

            def attn_scores_pair(b):
                p0, d0 = attn_scores(b, 0)
                p1, d1 = attn_scores(b, 1)
                return p0, d0, p1, d1

            def attn_tail_pair(b, a):
                p0, d0, p1, d1 = a
                attn_tail(b, 0, p0, d0)
                attn_tail(b, 1, p1, d1)

            def wo_block(sb, gs=range(8)):
                s0 = QB * sb
                for g in gs:                 # dtile pairs {2g, 2g+1}
                    yp = pj.tile([128, 2 * QB], F32,
                                 tag="qq" if g % 2 == 0 else "kv",
                                 name=f"yp{sb}_{g}")
                    for j in range(2):
                        d0 = 128 * (2 * g + j)
                        nc.tensor.matmul(yp[:, QB * j:QB * (j + 1)],
                                         wo_sb[:, d0:d0 + 128],
                                         OT0[:, s0:s0 + QB], start=True,
                                         stop=False)
                        nc.tensor.matmul(yp[:, QB * j:QB * (j + 1)],
                                         wo_sb[:, D + d0:D + d0 + 128],
                                         OT1[:, s0:s0 + QB], start=False,
                                         stop=True)
                    yts = tp.tile([128, 2 * QB], BF16, tag="yts", bufs=4,
                                  name=f"yts{sb}_{g}")
                    if g % 2 == 0:
                        nc.scalar.copy(yts[:], yp[:])
                    else:
                        nc.vector.tensor_copy(yts[:], yp[:])
                    c0 = (sb * NKT + 2 * g) * QB
                    nc.sync.dma_start(yT[:, c0:c0 + 2 * QB], yts[:])

            # ---- schedule: keep PE busy, hide DVE/scalar latency ----
            proj_sblock(0)
            proj_sblock(1)
            load_x(1)
            a0 = attn_scores_pair(0)
            proj_sblock(2)
            vtrans(0)
            attn_tail_pair(0, a0)
            a1 = attn_scores_pair(1)
            proj_sblock(3)
            vtrans(1)
            attn_tail_pair(1, a1)
            wo_block(0)
            a2 = attn_scores_pair(2)
            vtrans(2)
            attn_tail_pair(2, a2)
            wo_block(1)
            a3 = attn_scores_pair(3)
            vtrans(3)
            attn_tail_pair(3, a3)
            wo_block(2)
            wo_block(3)

    nc.compile()
    return nc


def _softplus(x):
    return np.log1p(np.exp(-np.abs(x))) + np.maximum(x, 0)


def _host_prep(inputs):
    import ml_dtypes
    bf = ml_dtypes.bfloat16
    x = np.asarray(inputs['x'], np.float32)
    freq = np.asarray(inputs['freq_cis'], np.float32)
    wq = np.asarray(inputs['wq'], np.float32)
    wk = np.asarray(inputs['wk'], np.float32)
    wv = np.asarray(inputs['wv'], np.float32)
    wo = np.asarray(inputs['wo'], np.float32)
    alk = np.asarray(inputs['a_log_k'], np.float32)
    bk = np.asarray(inputs['b_k'], np.float32)
    ck = np.asarray(inputs['c_k'], np.float32)
    alv = np.asarray(inputs['a_log_v'], np.float32)
    bv = np.asarray(inputs['b_v'], np.float32)
    cv = np.asarray(inputs['c_v'], np.float32)

    perm = np.concatenate([np.arange(0, HD, 2), np.arange(1, HD, 2)])
    xT = np.ascontiguousarray(x[0].T).astype(bf)            # (D, S)
    cos = np.ascontiguousarray(freq[:, :, 0, 0].T)          # (64, S)
    sin = np.ascontiguousarray(freq[:, :, 1, 0].T)
    cos2 = np.ascontiguousarray(np.concatenate([cos, cos], 0))
    sin2 = np.ascontiguousarray(np.concatenate([sin, -sin], 0))

    # ---- masks ----
    col = np.arange(QB)
    band = np.full((64, QB), NEG, np.float32)
    for r in range(64):
        band[r, 8 * r + 8:] = 0.0
    # G0 = [first4 | chunk0] for b=0: first4 key k valid iff k<=t;
    # chunk0 boundary banded.
    mG0b0 = np.full((128, QB), NEG, np.float32)
    for k in range(4):
        mG0b0[k, k:] = 0.0
    mG0b0[4:68] = band
    # G1 = [chunk1 | chunk2] for b=2: chunk1 all valid, chunk2 banded.
    mG1b2 = np.zeros((128, QB), np.float32)
    mG1b2[64:128] = band
    # G2 (b=3) and C1 (b=1): plain band in rows 0:64.
    mG2b3 = np.full((128, QB), NEG, np.float32)
    mG2b3[0:64] = band

    f32c = np.zeros((128, F32C_W), np.float32)
    f32c[:, MG0B0:MG0B0 + QB] = mG0b0
    f32c[:, MG1B2:MG1B2 + QB] = mG1b2
    f32c[:, MG2B3:MG2B3 + QB] = mG2b3
    f32c[0, ONESF:ONESF + 128] = 1.0

    t = np.arange(S)
    mdiag = np.where((t >= 5) & (t % 8 != 0), 0.0, NEG).astype(np.float32)
    mdiag = mdiag.reshape(4, QB)

    bf16c = np.zeros((128, BF16C_W), np.float32)
    bf16c[0, MDIAG:MDIAG + S] = mdiag.reshape(-1)
    bf16c[:, IDENT:IDENT + 128] = np.eye(128, dtype=np.float32)
    bf16c[:, ONES:ONES + 128] = 1.0

    ak_full = np.exp(-_softplus(alk.astype(np.float64))).astype(np.float32)
    av_full = np.exp(-_softplus(alv.astype(np.float64))).astype(np.float32)

    def pack_w(wslice):
        # (out_n, D) -> SBUF layout (128, NKT*out_n): [p, k*out_n + j]
        out_n = wslice.shape[0]
        wT = np.ascontiguousarray(wslice.T)  # (D, out_n)
        return np.ascontiguousarray(
            wT.reshape(NKT, 128, out_n).transpose(1, 0, 2).reshape(128, -1)
        ).astype(bf)

    shared = {
        "xT": xT, "cosk": cos2, "sink": sin2,
        "bf16c": bf16c.astype(bf),
    }

    in_maps = []
    for c in range(8):
        g = c // 2
        wq_c = wq[256 * c:256 * (c + 1)]
        wq_cp = np.concatenate(
            [wq_c[h * HD:(h + 1) * HD][perm] for h in range(2)])
        wk_g = wk[128 * g:128 * (g + 1)][perm]
        wv_g = wv[128 * g:128 * (g + 1)]
        ak = ak_full[128 * g:128 * (g + 1)][perm]
        bk_g = bk[128 * g:128 * (g + 1)][perm]
        ck_g = ck[128 * g:128 * (g + 1)][perm]
        av_ = av_full[128 * g:128 * (g + 1)]
        bv_g = bv[128 * g:128 * (g + 1)]
        cv_g = cv[128 * g:128 * (g + 1)]

        fcl = f32c.copy()
        fcl[:, AKP:AKP + QB] = np.where(col[None, :] % KC == 0, 0.0,
                                        ak[:, None])
        fcl[:, AVP:AVP + QB] = np.where(col[None, :] % KC == 0, 0.0,
                                        av_[:, None])
        fcl[:, CBK] = ck_g * bk_g
        fcl[:, CBV] = cv_g * bv_g

        wo_pack = np.concatenate(
            [np.ascontiguousarray(wo[:, 256 * c:256 * c + 128].T),
             np.ascontiguousarray(wo[:, 256 * c + 128:256 * (c + 1)].T)],
            axis=1).astype(bf)   # (128, 2*D)

        m = dict(shared)
        m.update({
            "wq": pack_w(wq_cp),
            "wk": pack_w(wk_g),
            "wv": pack_w(wv_g),
            "wo": wo_pack,
            "f32c": fcl,
        })
        in_maps.append(m)
    return in_maps


def kernel(**inputs) -> np.ndarray:
    if 'nc' not in _CACHE:
        _CACHE['nc'] = _build_module()
    nc = _CACHE['nc']
    in_maps = _host_prep(inputs)
    res = bass_utils.run_bass_kernel_spmd(nc, in_maps, core_ids=list(range(8)),
                                          **_CACHE.get('run_kwargs', {}))
    _CACHE['last_result'] = res
    acc = res.results[0]["yT"].astype(np.float64)
    for c in range(1, 8):
        acc += res.results[c]["yT"]
    # [p, (sb*16+dt)*512+c] -> y[128*dt+p, 512*sb+c] -> out (1, S, D)
    y = acc.reshape(128, 4, NKT, QB).transpose(2, 0, 1, 3).reshape(D, S)
    return np.ascontiguousarray(y.T[None]).astype(np.float32)
